# revision 1
# baseline (speedup 1.0000x reference)
"""Trainium2 Bass kernel for nn_AttentionBlock (B=4, C=H=W=S=256).

reference:
  q = Wq @ query + bq   (1x1 conv over channel dim)
  k = Wk @ key_in + bk
  v = Wv @ value + bv
  scores[b,i,h,w] = sum_j q[b,i,h,j] * k[b,j,i,w]
  attn = softmax(scores, -1)
  out[b,i,h,w] = sum_j attn[b,i,h,j] * v[b,i,j,w]
  return sigmoid(out)

Sharding: 8 cores = (b, g) with b=core//2, g=core%2; each core computes
out[b, g*128:(g+1)*128, :, :].

Per-core dataflow (i = local output channel, 128 of them):
  Phase A (streamed, DMA-bound):
    q[i, h, j] = WqT.T @ query[b]   (fp32r matmul)  -> q_scr DRAM fp16 [i,h,j]
    v[i, j, w] = WvT.T @ value[b]   (bf16 matmul)   -> v_scr DRAM bf16 [i,j,w]
  Phase B (per 16-i chunk):
    k[j, il, w] = WkT.T @ key_in[b][:, i_half, :]  (fp32r) -> SBUF fp16
    per i:
      qT_i [j, h]   <- DMA-transpose(q_scr[i])           (fp16)
      v_i  [j, 257] <- v_scr[i] with ones column          (bf16)
      scoresT [w, h] = k_i.T @ qT_i                       (fp16 matmul, PSUM)
      E^T [w, h] = exp(scoresT)                           (ACT -> bf16 SBUF)
      out_aug [h, 257] = E^T.T @ v_i  (col 256 = rowsums) (bf16 matmul, PSUM)
      out[h, w] = sigmoid(out_aug[:, :256] / rowsum)      (ACT, scale AP)
"""

import numpy as np

import concourse.bass as bass
import concourse.tile as tile
from concourse import bacc, mybir
from concourse.bass_utils import run_bass_kernel_spmd

C = 256
HALF = 128          # output channels per core
N_CORES = 8
ICHUNK = 16         # i values per phase-B chunk
A_CHUNK = 1024      # flattened spatial elems per phase-A chunk (4 rows)

_CACHE = {}


def build_nc():
    if "nc" in _CACHE:
        return _CACHE["nc"]
    f32 = mybir.dt.float32
    f32r = mybir.dt.float32r
    f16 = mybir.dt.float16
    bf16 = mybir.dt.bfloat16

    nc = bacc.Bacc("TRN2", target_bir_lowering=False, debug=False,
                   num_devices=N_CORES)

    query_b = nc.dram_tensor("query_b", [C, C, C], f32, kind="ExternalInput").ap()
    key_h = nc.dram_tensor("key_h", [C, HALF, C], f32, kind="ExternalInput").ap()
    value_b = nc.dram_tensor("value_b", [C, C, C], f32, kind="ExternalInput").ap()
    wqT = nc.dram_tensor("wqT", [C, HALF], f32, kind="ExternalInput").ap()
    wkT = nc.dram_tensor("wkT", [C, C], f32, kind="ExternalInput").ap()
    wvT = nc.dram_tensor("wvT", [C, HALF], f32, kind="ExternalInput").ap()
    bq_h = nc.dram_tensor("bq_h", [HALF, 1], f32, kind="ExternalInput").ap()
    bk_f = nc.dram_tensor("bk_f", [C, 1], f32, kind="ExternalInput").ap()
    bv_h = nc.dram_tensor("bv_h", [HALF, 1], f32, kind="ExternalInput").ap()
    out_b = nc.dram_tensor("out_b", [HALF, C, C], f32, kind="ExternalOutput").ap()

    q_scr = nc.dram_tensor("q_scr", [HALF, C, C], f16).ap()
    v_scr = nc.dram_tensor("v_scr", [HALF, C, C], bf16).ap()

    HJ = C * C  # 65536

    with tile.TileContext(nc) as tc:
        with tc.tile_pool(name="weights", bufs=1) as wpool:
            wq_r = wpool.tile([128, 2, HALF], f32r)
            wk_r = wpool.tile([128, 2, C], f32r)
            wv_bf = wpool.tile([128, 2, HALF], bf16)
            nc.gpsimd.dma_start(out=wq_r, in_=wqT.rearrange("(cb c) i -> c cb i", c=128))
            nc.gpsimd.dma_start(out=wk_r, in_=wkT.rearrange("(cb c) j -> c cb j", c=128))
            nc.gpsimd.dma_start(out=wv_bf, in_=wvT.rearrange("(cb c) i -> c cb i", c=128))
            sb_bq = wpool.tile([128, 1], f32)
            sb_bk = wpool.tile([128, 2, 1], f32)
            sb_bv = wpool.tile([128, 1], f32)
            nc.gpsimd.dma_start(out=sb_bq, in_=bq_h)
            nc.gpsimd.dma_start(out=sb_bk, in_=bk_f.rearrange("(jb j) one -> j jb one", j=128))
            nc.gpsimd.dma_start(out=sb_bv, in_=bv_h)

            # ---------------- Phase A: q and v convs -> DRAM scratch ----------
            qv_in = query_b.rearrange("(cb c) h j -> c cb (h j)", c=128)
            vv_in = value_b.rearrange("(cb c) j w -> c cb (j w)", c=128)
            q_flat = q_scr.rearrange("i h j -> i (h j)")
            v_flat = v_scr.rearrange("i j w -> i (j w)")
            n_chunks = HJ // A_CHUNK  # 64

            with (
                tc.tile_pool(name="a_in", bufs=3) as a_in,
                tc.tile_pool(name="a_st", bufs=3) as a_st,
                tc.tile_pool(name="a_ps", bufs=2, space="PSUM") as a_ps,
            ):
                for t in range(n_chunks):
                    sl = slice(t * A_CHUNK, (t + 1) * A_CHUNK)
                    # q conv chunk
                    qc = a_in.tile([128, 2, A_CHUNK], f32r, tag="qc")
                    nc.gpsimd.dma_start(out=qc, in_=qv_in[:, :, sl])
                    qs = a_st.tile([128, A_CHUNK], f16, tag="qs")
                    for n in range(A_CHUNK // 512):
                        ps = a_ps.tile([128, 512], f32, tag="aps")
                        for cb in range(2):
                            nc.tensor.matmul(ps, wq_r[:, cb, :],
                                             qc[:, cb, n * 512:(n + 1) * 512],
                                             start=(cb == 0), stop=(cb == 1))
                        nc.scalar.activation(out=qs[:, n * 512:(n + 1) * 512], in_=ps,
                                             func=mybir.ActivationFunctionType.Identity,
                                             bias=sb_bq)
                    nc.sync.dma_start(out=q_flat[:, sl], in_=qs)
                    # v conv chunk
                    vc = a_in.tile([128, 2, A_CHUNK], bf16, tag="vc")
                    nc.gpsimd.dma_start(out=vc, in_=vv_in[:, :, sl])
                    vs = a_st.tile([128, A_CHUNK], bf16, tag="vs")
                    for n in range(A_CHUNK // 512):
                        ps = a_ps.tile([128, 512], f32, tag="aps")
                        for cb in range(2):
                            nc.tensor.matmul(ps, wv_bf[:, cb, :],
                                             vc[:, cb, n * 512:(n + 1) * 512],
                                             start=(cb == 0), stop=(cb == 1))
                        nc.scalar.activation(out=vs[:, n * 512:(n + 1) * 512], in_=ps,
                                             func=mybir.ActivationFunctionType.Identity,
                                             bias=sb_bv)
                    nc.sync.dma_start(out=v_flat[:, sl], in_=vs)

            # ---------------- Phase B: k conv + attention ---------------------
            kv_in = key_h.rearrange("(cb c) il w -> c cb (il w)", c=128)
            out_v = out_b.rearrange("il (hb h) w -> il h hb w", h=128)

            with (
                tc.tile_pool(name="b_kin", bufs=2) as b_kin,
                tc.tile_pool(name="b_ksb", bufs=2) as b_ksb,
                tc.tile_pool(name="b_qt", bufs=2) as b_qt,
                tc.tile_pool(name="b_vt", bufs=3) as b_vt,
                tc.tile_pool(name="b_et", bufs=3) as b_et,
                tc.tile_pool(name="b_ob", bufs=3) as b_ob,
                tc.tile_pool(name="b_rs", bufs=6) as b_rs,
                tc.tile_pool(name="b_psk", bufs=2, space="PSUM") as b_psk,
                tc.tile_pool(name="b_psc", bufs=2, space="PSUM") as b_psc,
                tc.tile_pool(name="b_po", bufs=4, space="PSUM") as b_po,
            ):
                KCH = ICHUNK * C  # 4096 flattened (il, w) per chunk
                for ic in range(HALF // ICHUNK):  # 8 chunks
                    kc = b_kin.tile([128, 2, KCH], f32r, tag="kc")
                    nc.gpsimd.dma_start(
                        out=kc, in_=kv_in[:, :, ic * KCH:(ic + 1) * KCH])
                    ksb = b_ksb.tile([128, 2, ICHUNK, C], f16, tag="ksb")
                    for jb in range(2):
                        for n in range(KCH // 512):
                            ps = b_psk.tile([128, 512], f32, tag="psk")
                            for cb in range(2):
                                nc.tensor.matmul(
                                    ps, wk_r[:, cb, jb * 128:(jb + 1) * 128],
                                    kc[:, cb, n * 512:(n + 1) * 512],
                                    start=(cb == 0), stop=(cb == 1))
                            nc.vector.tensor_scalar(
                                out=ksb[:, jb, n * 2:(n + 1) * 2, :], in0=ps,
                                scalar1=sb_bk[:, jb, :], scalar2=None,
                                op0=mybir.AluOpType.add)
                    # batch the 32 transpose reads for this chunk
                    qt = b_qt.tile([128, ICHUNK, 2, C], f16, tag="qt")
                    for t in range(ICHUNK):
                        i_loc = ic * ICHUNK + t
                        for jb in range(2):
                            nc.sync.dma_start(
                                out=qt[:, t, jb, :],
                                in_=q_scr[i_loc, :, jb * 128:(jb + 1) * 128],
                                transpose=True)
                    for t in range(ICHUNK):
                        i_loc = ic * ICHUNK + t
                        vt = b_vt.tile([128, 2, C + 1], bf16, tag="vt")
                        nc.sync.dma_start(
                            out=vt[:, :, 0:C],
                            in_=v_scr[i_loc].rearrange("(jb j) w -> j jb w", j=128))
                        nc.vector.memset(vt[:, :, C:C + 1], 1.0)
                        psc = b_psc.tile([128, 2, C], f32, tag="psc")
                        for wb in range(2):
                            for jb in range(2):
                                nc.tensor.matmul(
                                    psc[:, wb, :],
                                    ksb[:, jb, t, wb * 128:(wb + 1) * 128],
                                    qt[:, t, jb, :],
                                    start=(jb == 0), stop=(jb == 1))
                        et = b_et.tile([128, 2, C], bf16, tag="et")
                        for wb in range(2):
                            nc.scalar.activation(
                                out=et[:, wb, :], in_=psc[:, wb, :],
                                func=mybir.ActivationFunctionType.Exp)
                        ob = b_ob.tile([128, 2, C], f32, tag="ob")
                        for hb in range(2):
                            po = b_po.tile([128, C + 1], f32, tag="po")
                            for wb in range(2):
                                nc.tensor.matmul(
                                    po, et[:, wb, hb * 128:(hb + 1) * 128],
                                    vt[:, wb, :],
                                    start=(wb == 0), stop=(wb == 1))
                            rs = b_rs.tile([128, 1], f32, tag="rs")
                            nc.vector.reciprocal(out=rs, in_=po[:, C:C + 1])
                            nc.scalar.activation(
                                out=ob[:, hb, :], in_=po[:, 0:C],
                                func=mybir.ActivationFunctionType.Sigmoid,
                                scale=rs)
                        nc.sync.dma_start(out=out_v[i_loc], in_=ob)

    nc.compile()
    _CACHE["nc"] = nc
    return nc


def kernel(query, key_in, value, Wq, bq, Wk, bk, Wv, bv):
    query = np.ascontiguousarray(query, dtype=np.float32)
    key_in = np.ascontiguousarray(key_in, dtype=np.float32)
    value = np.ascontiguousarray(value, dtype=np.float32)
    Wq = np.asarray(Wq, dtype=np.float32)
    Wk = np.asarray(Wk, dtype=np.float32)
    Wv = np.asarray(Wv, dtype=np.float32)
    bq = np.asarray(bq, dtype=np.float32)
    bk = np.asarray(bk, dtype=np.float32)
    bv = np.asarray(bv, dtype=np.float32)

    nc = build_nc()
    in_maps = []
    for core in range(N_CORES):
        b, g = core // 2, core % 2
        sl = slice(g * HALF, (g + 1) * HALF)
        in_maps.append({
            "query_b": query[b],
            "key_h": np.ascontiguousarray(key_in[b][:, sl, :]),
            "value_b": value[b],
            "wqT": np.ascontiguousarray(Wq[sl, :].T),
            "wkT": np.ascontiguousarray(Wk.T),
            "wvT": np.ascontiguousarray(Wv[sl, :].T),
            "bq_h": np.ascontiguousarray(bq[sl].reshape(HALF, 1)),
            "bk_f": np.ascontiguousarray(bk.reshape(C, 1)),
            "bv_h": np.ascontiguousarray(bv[sl].reshape(HALF, 1)),
        })
    res = run_bass_kernel_spmd(nc, in_maps, core_ids=list(range(N_CORES)))
    out = np.empty((4, C, C, C), dtype=np.float32)
    for core in range(N_CORES):
        b, g = core // 2, core % 2
        out[b, g * HALF:(g + 1) * HALF] = res.results[core]["out_b"]
    return out


# revision 2
# speedup vs baseline: 15590.6348x; 15590.6348x over previous
"""Trainium2 Bass kernel for nn_AttentionBlock (B=4, C=H=W=S=256).

reference:
  q = Wq @ query + bq   (1x1 conv over channel dim)
  k = Wk @ key_in + bk
  v = Wv @ value + bv
  scores[b,i,h,w] = sum_j q[b,i,h,j] * k[b,j,i,w]
  attn = softmax(scores, -1)
  out[b,i,h,w] = sum_j attn[b,i,h,j] * v[b,i,j,w]
  return sigmoid(out)

Sharding: 8 cores = (b, g) with b=core//2, g=core%2; each core computes
out[b, g*128:(g+1)*128, :, :].

Per-core dataflow (i = local output channel, 128 of them):
  Phase A (streamed, DMA-bound):
    q[i, h, j] = WqT.T @ query[b]   (fp32r matmul)  -> q_scr DRAM fp16 [i,h,j]
    v[i, j, w] = WvT.T @ value[b]   (bf16 matmul)   -> v_scr DRAM bf16 [i,j,w]
  Phase B (per 16-i chunk):
    k[j, il, w] = WkT.T @ key_in[b][:, i_half, :]  (fp32r) -> SBUF fp16
    per i:
      qT_i [j, h]   <- DMA-transpose(q_scr[i])           (fp16)
      v_i  [j, 257] <- v_scr[i] with ones column          (bf16)
      scoresT [w, h] = k_i.T @ qT_i                       (fp16 matmul, PSUM)
      E^T [w, h] = exp(scoresT)                           (ACT -> bf16 SBUF)
      out_aug [h, 257] = E^T.T @ v_i  (col 256 = rowsums) (bf16 matmul, PSUM)
      out[h, w] = sigmoid(out_aug[:, :256] / rowsum)      (ACT, scale AP)
"""

import numpy as np

import concourse.bass as bass
import concourse.tile as tile
from concourse import bacc, mybir
from concourse.bass_utils import run_bass_kernel_spmd

C = 256
HALF = 128          # output channels per core
N_CORES = 8
ICHUNK = 16         # i values per phase-B chunk
A_CHUNK = 1024      # flattened spatial elems per phase-A chunk (4 rows)

_CACHE = {}


def build_nc():
    if "nc" in _CACHE:
        return _CACHE["nc"]
    f32 = mybir.dt.float32
    f32r = mybir.dt.float32r
    f16 = mybir.dt.float16
    bf16 = mybir.dt.bfloat16

    nc = bacc.Bacc("TRN2", target_bir_lowering=False, debug=False,
                   num_devices=N_CORES)

    query_b = nc.dram_tensor("query_b", [C, C, C], f32, kind="ExternalInput").ap()
    key_h = nc.dram_tensor("key_h", [C, HALF, C], f32, kind="ExternalInput").ap()
    value_b = nc.dram_tensor("value_b", [C, C, C], f32, kind="ExternalInput").ap()
    wqT = nc.dram_tensor("wqT", [C, HALF], f32, kind="ExternalInput").ap()
    wkT = nc.dram_tensor("wkT", [C, C], f32, kind="ExternalInput").ap()
    wvT = nc.dram_tensor("wvT", [C, HALF], f32, kind="ExternalInput").ap()
    bq_h = nc.dram_tensor("bq_h", [HALF, 1], f32, kind="ExternalInput").ap()
    bk_f = nc.dram_tensor("bk_f", [C, 1], f32, kind="ExternalInput").ap()
    bv_h = nc.dram_tensor("bv_h", [HALF, 1], f32, kind="ExternalInput").ap()
    out_b = nc.dram_tensor("out_b", [HALF, C, C], f32, kind="ExternalOutput").ap()

    q_scr = nc.dram_tensor("q_scr", [HALF, C, C], f16).ap()
    v_scr = nc.dram_tensor("v_scr", [HALF, C, C], bf16).ap()

    HJ = C * C  # 65536

    with tile.TileContext(nc) as tc:
        with tc.tile_pool(name="weights", bufs=1) as wpool:
            wq_r = wpool.tile([128, 2, HALF], f32r)
            wk_r = wpool.tile([128, 2, C], f32r)
            wv_bf = wpool.tile([128, 2, HALF], bf16)
            nc.gpsimd.dma_start(out=wq_r, in_=wqT.rearrange("(cb c) i -> c cb i", c=128))
            nc.gpsimd.dma_start(out=wk_r, in_=wkT.rearrange("(cb c) j -> c cb j", c=128))
            nc.gpsimd.dma_start(out=wv_bf, in_=wvT.rearrange("(cb c) i -> c cb i", c=128))
            sb_bq = wpool.tile([128, 1], f32)
            sb_bk = wpool.tile([128, 2, 1], f32)
            sb_bv = wpool.tile([128, 1], f32)
            nc.gpsimd.dma_start(out=sb_bq, in_=bq_h)
            nc.gpsimd.dma_start(out=sb_bk, in_=bk_f.rearrange("(jb j) one -> j jb one", j=128))
            nc.gpsimd.dma_start(out=sb_bv, in_=bv_h)

            # ---------------- Phase A: q and v convs -> DRAM scratch ----------
            qv_in = query_b.rearrange("(cb c) h j -> c cb (h j)", c=128)
            vv_in = value_b.rearrange("(cb c) j w -> c cb (j w)", c=128)
            q_flat = q_scr.rearrange("i h j -> i (h j)")
            v_flat = v_scr.rearrange("i j w -> i (j w)")
            n_chunks = HJ // A_CHUNK  # 64

            with (
                tc.tile_pool(name="a_in", bufs=3) as a_in,
                tc.tile_pool(name="a_st", bufs=3) as a_st,
                tc.tile_pool(name="a_ps", bufs=2, space="PSUM") as a_ps,
            ):
                for t in range(n_chunks):
                    sl = slice(t * A_CHUNK, (t + 1) * A_CHUNK)
                    # q conv chunk
                    qc = a_in.tile([128, 2, A_CHUNK], f32r, tag="qc")
                    nc.gpsimd.dma_start(out=qc, in_=qv_in[:, :, sl])
                    qs = a_st.tile([128, A_CHUNK], f16, tag="qs")
                    for n in range(A_CHUNK // 512):
                        ps = a_ps.tile([128, 512], f32, tag="aps")
                        for cb in range(2):
                            nc.tensor.matmul(ps, wq_r[:, cb, :],
                                             qc[:, cb, n * 512:(n + 1) * 512],
                                             start=(cb == 0), stop=(cb == 1))
                        nc.scalar.activation(out=qs[:, n * 512:(n + 1) * 512], in_=ps,
                                             func=mybir.ActivationFunctionType.Identity,
                                             bias=sb_bq)
                    nc.sync.dma_start(out=q_flat[:, sl], in_=qs)
                    # v conv chunk
                    vc = a_in.tile([128, 2, A_CHUNK], bf16, tag="vc")
                    nc.gpsimd.dma_start(out=vc, in_=vv_in[:, :, sl])
                    vs = a_st.tile([128, A_CHUNK], bf16, tag="vs")
                    for n in range(A_CHUNK // 512):
                        ps = a_ps.tile([128, 512], f32, tag="aps")
                        for cb in range(2):
                            nc.tensor.matmul(ps, wv_bf[:, cb, :],
                                             vc[:, cb, n * 512:(n + 1) * 512],
                                             start=(cb == 0), stop=(cb == 1))
                        nc.scalar.activation(out=vs[:, n * 512:(n + 1) * 512], in_=ps,
                                             func=mybir.ActivationFunctionType.Identity,
                                             bias=sb_bv)
                    nc.sync.dma_start(out=v_flat[:, sl], in_=vs)

            # ---------------- Phase B: k conv + attention ---------------------
            kv_in = key_h.rearrange("(cb c) il w -> c cb (il w)", c=128)
            out_v = out_b.rearrange("il (hb h) w -> il h hb w", h=128)

            with (
                tc.tile_pool(name="b_kin", bufs=2) as b_kin,
                tc.tile_pool(name="b_ksb", bufs=2) as b_ksb,
                tc.tile_pool(name="b_qt", bufs=2) as b_qt,
                tc.tile_pool(name="b_vt", bufs=3) as b_vt,
                tc.tile_pool(name="b_et", bufs=3) as b_et,
                tc.tile_pool(name="b_ob", bufs=3) as b_ob,
                tc.tile_pool(name="b_rs", bufs=6) as b_rs,
                tc.tile_pool(name="b_psk", bufs=2, space="PSUM") as b_psk,
                tc.tile_pool(name="b_psc", bufs=2, space="PSUM") as b_psc,
                tc.tile_pool(name="b_po", bufs=4, space="PSUM") as b_po,
            ):
                KCH = ICHUNK * C  # 4096 flattened (il, w) per chunk
                for ic in range(HALF // ICHUNK):  # 8 chunks
                    kc = b_kin.tile([128, 2, KCH], f32r, tag="kc")
                    nc.gpsimd.dma_start(
                        out=kc, in_=kv_in[:, :, ic * KCH:(ic + 1) * KCH])
                    ksb = b_ksb.tile([128, 2, ICHUNK, C], f16, tag="ksb")
                    for jb in range(2):
                        for n in range(KCH // 512):
                            ps = b_psk.tile([128, 512], f32, tag="psk")
                            for cb in range(2):
                                nc.tensor.matmul(
                                    ps, wk_r[:, cb, jb * 128:(jb + 1) * 128],
                                    kc[:, cb, n * 512:(n + 1) * 512],
                                    start=(cb == 0), stop=(cb == 1))
                            nc.vector.tensor_scalar(
                                out=ksb[:, jb, n * 2:(n + 1) * 2, :], in0=ps,
                                scalar1=sb_bk[:, jb, :], scalar2=None,
                                op0=mybir.AluOpType.add)
                    # batch the 32 transpose reads for this chunk
                    qt = b_qt.tile([128, ICHUNK, 2, C], f16, tag="qt")
                    for t in range(ICHUNK):
                        i_loc = ic * ICHUNK + t
                        for jb in range(2):
                            nc.sync.dma_start(
                                out=qt[:, t, jb, :],
                                in_=q_scr[i_loc, :, jb * 128:(jb + 1) * 128],
                                transpose=True)
                    for t in range(ICHUNK):
                        i_loc = ic * ICHUNK + t
                        vt = b_vt.tile([128, 2, C + 1], bf16, tag="vt")
                        nc.sync.dma_start(
                            out=vt[:, :, 0:C],
                            in_=v_scr[i_loc].rearrange("(jb j) w -> j jb w", j=128))
                        nc.vector.memset(vt[:, :, C:C + 1], 1.0)
                        psc = b_psc.tile([128, 2, C], f32, tag="psc")
                        for wb in range(2):
                            for jb in range(2):
                                nc.tensor.matmul(
                                    psc[:, wb, :],
                                    ksb[:, jb, t, wb * 128:(wb + 1) * 128],
                                    qt[:, t, jb, :],
                                    start=(jb == 0), stop=(jb == 1))
                        et = b_et.tile([128, 2, C], bf16, tag="et")
                        for wb in range(2):
                            nc.scalar.activation(
                                out=et[:, wb, :], in_=psc[:, wb, :],
                                func=mybir.ActivationFunctionType.Exp)
                        ob = b_ob.tile([128, 2, C], f32, tag="ob")
                        for hb in range(2):
                            po = b_po.tile([128, C + 1], f32, tag="po")
                            for wb in range(2):
                                nc.tensor.matmul(
                                    po, et[:, wb, hb * 128:(hb + 1) * 128],
                                    vt[:, wb, :],
                                    start=(wb == 0), stop=(wb == 1))
                            rs = b_rs.tile([128, 1], f32, tag="rs")
                            nc.vector.reciprocal(out=rs, in_=po[:, C:C + 1])
                            nc.scalar.activation(
                                out=ob[:, hb, :], in_=po[:, 0:C],
                                func=mybir.ActivationFunctionType.Sigmoid,
                                scale=rs)
                        nc.sync.dma_start(out=out_v[i_loc], in_=ob)

    nc.compile()
    _CACHE["nc"] = nc
    return nc


def make_in_maps(inputs):
    query = np.ascontiguousarray(inputs["query"], dtype=np.float32)
    key_in = np.ascontiguousarray(inputs["key_in"], dtype=np.float32)
    value = np.ascontiguousarray(inputs["value"], dtype=np.float32)
    Wq = np.asarray(inputs["Wq"], dtype=np.float32)
    Wk = np.asarray(inputs["Wk"], dtype=np.float32)
    Wv = np.asarray(inputs["Wv"], dtype=np.float32)
    bq = np.asarray(inputs["bq"], dtype=np.float32)
    bk = np.asarray(inputs["bk"], dtype=np.float32)
    bv = np.asarray(inputs["bv"], dtype=np.float32)
    in_maps = []
    for core in range(N_CORES):
        b, g = core // 2, core % 2
        sl = slice(g * HALF, (g + 1) * HALF)
        in_maps.append({
            "query_b": query[b],
            "key_h": np.ascontiguousarray(key_in[b][:, sl, :]),
            "value_b": value[b],
            "wqT": np.ascontiguousarray(Wq[sl, :].T),
            "wkT": np.ascontiguousarray(Wk.T),
            "wvT": np.ascontiguousarray(Wv[sl, :].T),
            "bq_h": np.ascontiguousarray(bq[sl].reshape(HALF, 1)),
            "bk_f": np.ascontiguousarray(bk.reshape(C, 1)),
            "bv_h": np.ascontiguousarray(bv[sl].reshape(HALF, 1)),
        })
    return in_maps


def kernel(query, key_in, value, Wq, bq, Wk, bk, Wv, bv):
    nc = build_nc()
    in_maps = make_in_maps(dict(query=query, key_in=key_in, value=value,
                                Wq=Wq, bq=bq, Wk=Wk, bk=bk, Wv=Wv, bv=bv))
    res = run_bass_kernel_spmd(nc, in_maps, core_ids=list(range(N_CORES)))
    out = np.empty((4, C, C, C), dtype=np.float32)
    for core in range(N_CORES):
        b, g = core // 2, core % 2
        out[b, g * HALF:(g + 1) * HALF] = res.results[core]["out_b"]
    return out


# revision 3
# speedup vs baseline: 28631.0215x; 1.8364x over previous
"""Trainium2 Bass kernel for nn_AttentionBlock (B=4, C=H=W=S=256).

reference:
  q = Wq @ query + bq   (1x1 conv over channel dim)
  k = Wk @ key_in + bk
  v = Wv @ value + bv
  scores[b,i,h,w] = sum_j q[b,i,h,j] * k[b,j,i,w]
  attn = softmax(scores, -1)
  out[b,i,h,w] = sum_j attn[b,i,h,j] * v[b,i,j,w]
  return sigmoid(out)

Sharding: 8 cores = (b, g) with b=core//2, g=core%2; each core computes
out[b, g*128:(g+1)*128, :, :].

Per-core dataflow (i = local output channel, 128 of them):
  Phase A (streamed, DMA-bound):
    q[i, h, j] = WqT.T @ query[b]   (fp32r matmul)  -> q_scr DRAM fp16 [i,h,j]
    v[i, j, w] = WvT.T @ value[b]   (bf16 matmul)   -> v_scr DRAM bf16 [i,j,w]
  Phase B (per ICHUNK of i):
    k[j, il, w] = WkT.T @ key_in[b][:, i_half, :]  (fp32r) -> SBUF fp16
    per i:
      qT_i [j, h]   <- DMA-transpose(q_scr[i])           (fp16)
      v_i  [j, 257] <- v_scr[i] with ones column          (bf16)
      scoresT [w, h] = k_i.T @ qT_i                       (fp16 matmul, PSUM)
      E^T [w, h] = exp(scoresT)                           (ACT -> bf16 SBUF)
      out_aug [h, 257] = E^T.T @ v_i  (col 256 = rowsums) (bf16 matmul, PSUM)
      out[h, w] = sigmoid(out_aug[:, :256] / rowsum)      (ACT, scale AP)
"""

import numpy as np

import concourse.bass as bass
import concourse.tile as tile
from concourse import bacc, mybir
from concourse.bass_utils import run_bass_kernel_spmd

C = 256
HALF = 128          # output channels per core
N_CORES = 8
ICHUNK = 8          # i values per phase-B chunk
A_CHUNK = 1024      # flattened spatial elems per phase-A chunk

_CACHE = {}


def build_nc(repeat=1):
    key = ("nc", repeat)
    if key in _CACHE:
        return _CACHE[key]
    f32 = mybir.dt.float32
    f32r = mybir.dt.float32r
    f16 = mybir.dt.float16
    bf16 = mybir.dt.bfloat16
    Ident = mybir.ActivationFunctionType.Identity

    nc = bacc.Bacc("TRN2", target_bir_lowering=False, debug=False,
                   num_devices=N_CORES)

    query_b = nc.dram_tensor("query_b", [C, C, C], f32, kind="ExternalInput").ap()
    key_h = nc.dram_tensor("key_h", [C, HALF, C], f32, kind="ExternalInput").ap()
    value_b = nc.dram_tensor("value_b", [C, C, C], f32, kind="ExternalInput").ap()
    wqT = nc.dram_tensor("wqT", [C, HALF], f32, kind="ExternalInput").ap()
    wkT = nc.dram_tensor("wkT", [C, C], f32, kind="ExternalInput").ap()
    wvT = nc.dram_tensor("wvT", [C, HALF], f32, kind="ExternalInput").ap()
    bq_h = nc.dram_tensor("bq_h", [HALF, 1], f32, kind="ExternalInput").ap()
    bk_f = nc.dram_tensor("bk_f", [C, 1], f32, kind="ExternalInput").ap()
    bv_h = nc.dram_tensor("bv_h", [HALF, 1], f32, kind="ExternalInput").ap()
    out_b = nc.dram_tensor("out_b", [HALF, C, C], f32, kind="ExternalOutput").ap()

    q_scr = nc.dram_tensor("q_scr", [HALF, C, C], f16).ap()
    v_scr = nc.dram_tensor("v_scr", [HALF, C, C], bf16).ap()

    HJ = C * C  # 65536
    KCH = ICHUNK * C  # flattened (il, w) per phase-B chunk

    qv_in = query_b.rearrange("(cb c) h j -> c cb (h j)", c=128)
    vv_in = value_b.rearrange("(cb c) j w -> c cb (j w)", c=128)
    kv_in = key_h.rearrange("(cb c) il w -> c cb (il w)", c=128)
    q_flat = q_scr.rearrange("i h j -> i (h j)")
    v_flat = v_scr.rearrange("i j w -> i (j w)")
    out_v = out_b.rearrange("il (hb h) w -> il h hb w", h=128)

    with tile.TileContext(nc) as tc:
        with (
            tc.tile_pool(name="weights", bufs=1) as wpool,
            tc.tile_pool(name="a_in", bufs=3) as a_in,
            tc.tile_pool(name="a_st", bufs=3) as a_st,
            tc.tile_pool(name="ps512", bufs=2, space="PSUM") as ps512,
            tc.tile_pool(name="b_kin", bufs=2) as b_kin,
            tc.tile_pool(name="b_ksb", bufs=2) as b_ksb,
            tc.tile_pool(name="b_qt", bufs=2) as b_qt,
            tc.tile_pool(name="b_vt", bufs=3) as b_vt,
            tc.tile_pool(name="b_et", bufs=3) as b_et,
            tc.tile_pool(name="b_ob", bufs=3) as b_ob,
            tc.tile_pool(name="b_rs", bufs=6) as b_rs,
            tc.tile_pool(name="b_psc", bufs=2, space="PSUM") as b_psc,
            tc.tile_pool(name="b_po", bufs=4, space="PSUM") as b_po,
        ):
            wq_r = wpool.tile([128, 2, HALF], f32r)
            wk_r = wpool.tile([128, 2, C], f32r)
            wv_bf = wpool.tile([128, 2, HALF], bf16)
            nc.gpsimd.dma_start(out=wq_r, in_=wqT.rearrange("(cb c) i -> c cb i", c=128))
            nc.gpsimd.dma_start(out=wk_r, in_=wkT.rearrange("(cb c) j -> c cb j", c=128))
            nc.gpsimd.dma_start(out=wv_bf, in_=wvT.rearrange("(cb c) i -> c cb i", c=128))
            sb_bq = wpool.tile([128, 1], f32)
            sb_bk = wpool.tile([128, 2, 1], f32)
            sb_bv = wpool.tile([128, 1], f32)
            nc.gpsimd.dma_start(out=sb_bq, in_=bq_h)
            nc.gpsimd.dma_start(out=sb_bk, in_=bk_f.rearrange("(jb j) one -> j jb one", j=128))
            nc.gpsimd.dma_start(out=sb_bv, in_=bv_h)

            def body(_it=None):
                # ---------------- Phase A: q and v convs -> DRAM scratch ------
                for t in range(HJ // A_CHUNK):
                    sl = slice(t * A_CHUNK, (t + 1) * A_CHUNK)
                    qc = a_in.tile([128, 2, A_CHUNK], f32r, tag="qc")
                    nc.gpsimd.dma_start(out=qc, in_=qv_in[:, :, sl])
                    qs = a_st.tile([128, A_CHUNK], f16, tag="qs")
                    for n in range(A_CHUNK // 512):
                        ps = ps512.tile([128, 512], f32, tag="aps")
                        for cb in range(2):
                            nc.tensor.matmul(ps, wq_r[:, cb, :],
                                             qc[:, cb, n * 512:(n + 1) * 512],
                                             start=(cb == 0), stop=(cb == 1))
                        nc.scalar.activation(out=qs[:, n * 512:(n + 1) * 512], in_=ps,
                                             func=Ident, bias=sb_bq)
                    nc.sync.dma_start(out=q_flat[:, sl], in_=qs)
                    vc = a_in.tile([128, 2, A_CHUNK], bf16, tag="vc")
                    nc.gpsimd.dma_start(out=vc, in_=vv_in[:, :, sl])
                    vs = a_st.tile([128, A_CHUNK], bf16, tag="vs")
                    for n in range(A_CHUNK // 512):
                        ps = ps512.tile([128, 512], f32, tag="aps")
                        for cb in range(2):
                            nc.tensor.matmul(ps, wv_bf[:, cb, :],
                                             vc[:, cb, n * 512:(n + 1) * 512],
                                             start=(cb == 0), stop=(cb == 1))
                        nc.scalar.activation(out=vs[:, n * 512:(n + 1) * 512], in_=ps,
                                             func=Ident, bias=sb_bv)
                    nc.sync.dma_start(out=v_flat[:, sl], in_=vs)

                # ---------------- Phase B: k conv + attention -----------------
                for ic in range(HALF // ICHUNK):
                    kc = b_kin.tile([128, 2, KCH], f32r, tag="kc")
                    nc.gpsimd.dma_start(
                        out=kc, in_=kv_in[:, :, ic * KCH:(ic + 1) * KCH])
                    ksb = b_ksb.tile([128, 2, ICHUNK, C], f16, tag="ksb")
                    for jb in range(2):
                        for n in range(KCH // 512):
                            ps = ps512.tile([128, 512], f32, tag="aps")
                            for cb in range(2):
                                nc.tensor.matmul(
                                    ps, wk_r[:, cb, jb * 128:(jb + 1) * 128],
                                    kc[:, cb, n * 512:(n + 1) * 512],
                                    start=(cb == 0), stop=(cb == 1))
                            nc.vector.tensor_scalar(
                                out=ksb[:, jb, n * 2:(n + 1) * 2, :], in0=ps,
                                scalar1=sb_bk[:, jb, :], scalar2=None,
                                op0=mybir.AluOpType.add)
                    qt = b_qt.tile([128, ICHUNK, 2, C], f16, tag="qt")
                    for t in range(ICHUNK):
                        i_loc = ic * ICHUNK + t
                        for jb in range(2):
                            nc.sync.dma_start(
                                out=qt[:, t, jb, :],
                                in_=q_scr[i_loc, :, jb * 128:(jb + 1) * 128],
                                transpose=True)
                    for t in range(ICHUNK):
                        i_loc = ic * ICHUNK + t
                        vt = b_vt.tile([128, 2, C + 1], bf16, tag="vt")
                        nc.sync.dma_start(
                            out=vt[:, :, 0:C],
                            in_=v_scr[i_loc].rearrange("(jb j) w -> j jb w", j=128))
                        nc.vector.memset(vt[:, :, C:C + 1], 1.0)
                        psc = b_psc.tile([128, 2, C], f32, tag="psc")
                        for wb in range(2):
                            for jb in range(2):
                                nc.tensor.matmul(
                                    psc[:, wb, :],
                                    ksb[:, jb, t, wb * 128:(wb + 1) * 128],
                                    qt[:, t, jb, :],
                                    start=(jb == 0), stop=(jb == 1))
                        et = b_et.tile([128, 2, C], bf16, tag="et")
                        for wb in range(2):
                            nc.scalar.activation(
                                out=et[:, wb, :], in_=psc[:, wb, :],
                                func=mybir.ActivationFunctionType.Exp)
                        ob = b_ob.tile([128, 2, C], f32, tag="ob")
                        for hb in range(2):
                            po = b_po.tile([128, C + 1], f32, tag="po")
                            for wb in range(2):
                                nc.tensor.matmul(
                                    po, et[:, wb, hb * 128:(hb + 1) * 128],
                                    vt[:, wb, :],
                                    start=(wb == 0), stop=(wb == 1))
                            rs = b_rs.tile([128, 1], f32, tag="rs")
                            nc.vector.reciprocal(out=rs, in_=po[:, C:C + 1])
                            nc.scalar.activation(
                                out=ob[:, hb, :], in_=po[:, 0:C],
                                func=mybir.ActivationFunctionType.Sigmoid,
                                scale=rs)
                        nc.sync.dma_start(out=out_v[i_loc], in_=ob)

            if repeat == 1:
                body()
            else:
                with tc.For_i(0, repeat, 1) as it:
                    body(it)

    nc.compile()
    _CACHE[key] = nc
    return nc


def make_in_maps(inputs):
    query = np.ascontiguousarray(inputs["query"], dtype=np.float32)
    key_in = np.ascontiguousarray(inputs["key_in"], dtype=np.float32)
    value = np.ascontiguousarray(inputs["value"], dtype=np.float32)
    Wq = np.asarray(inputs["Wq"], dtype=np.float32)
    Wk = np.asarray(inputs["Wk"], dtype=np.float32)
    Wv = np.asarray(inputs["Wv"], dtype=np.float32)
    bq = np.asarray(inputs["bq"], dtype=np.float32)
    bk = np.asarray(inputs["bk"], dtype=np.float32)
    bv = np.asarray(inputs["bv"], dtype=np.float32)
    in_maps = []
    for core in range(N_CORES):
        b, g = core // 2, core % 2
        sl = slice(g * HALF, (g + 1) * HALF)
        in_maps.append({
            "query_b": query[b],
            "key_h": np.ascontiguousarray(key_in[b][:, sl, :]),
            "value_b": value[b],
            "wqT": np.ascontiguousarray(Wq[sl, :].T),
            "wkT": np.ascontiguousarray(Wk.T),
            "wvT": np.ascontiguousarray(Wv[sl, :].T),
            "bq_h": np.ascontiguousarray(bq[sl].reshape(HALF, 1)),
            "bk_f": np.ascontiguousarray(bk.reshape(C, 1)),
            "bv_h": np.ascontiguousarray(bv[sl].reshape(HALF, 1)),
        })
    return in_maps


def kernel(query, key_in, value, Wq, bq, Wk, bk, Wv, bv):
    nc = build_nc()
    in_maps = make_in_maps(dict(query=query, key_in=key_in, value=value,
                                Wq=Wq, bq=bq, Wk=Wk, bk=bk, Wv=Wv, bv=bv))
    res = run_bass_kernel_spmd(nc, in_maps, core_ids=list(range(N_CORES)))
    out = np.empty((4, C, C, C), dtype=np.float32)
    for core in range(N_CORES):
        b, g = core // 2, core % 2
        out[b, g * HALF:(g + 1) * HALF] = res.results[core]["out_b"]
    return out


# revision 9
# speedup vs baseline: 53745.7026x; 1.8772x over previous
"""Trainium2 Bass kernel for nn_AttentionBlock (B=4, C=H=W=S=256).

reference:
  q = Wq @ query + bq   (1x1 conv over channel dim)
  k = Wk @ key_in + bk
  v = Wv @ value + bv
  scores[b,i,h,w] = sum_j q[b,i,h,j] * k[b,j,i,w]
  attn = softmax(scores, -1)
  out[b,i,h,w] = sum_j attn[b,i,h,j] * v[b,i,j,w]
  return sigmoid(out)

Sharding: 8 cores = (b, g) with b=core//2, g=core%2; each core computes
out[b, g*128:(g+1)*128, :, :].

Per-core dataflow (i = local output channel, 128 of them):
  Phase A (streamed, DMA-bound):
    q[i, h, j] = WqT.T @ query[b]   (fp32r matmul)  -> q_scr DRAM fp16 [i,h,j]
    v[i, j, w] = WvT.T @ value[b]   (bf16 matmul)   -> v_scr DRAM bf16 [i,j,w]
  Phase B (per ICHUNK of i):
    k[j, il, w] = WkT.T @ key_in[b][:, i_half, :]  (fp32r) -> SBUF fp16
    per i:
      qT_i [j, h]   <- DMA-transpose(q_scr[i])           (fp16)
      v_i  [j, 257] <- v_scr[i] with ones column          (bf16)
      scoresT [w, h] = k_i.T @ qT_i                       (fp16 matmul, PSUM)
      E^T [w, h] = exp(scoresT)                           (ACT -> bf16 SBUF)
      out_aug [h, 257] = E^T.T @ v_i  (col 256 = rowsums) (bf16 matmul, PSUM)
      out[h, w] = sigmoid(out_aug[:, :256] / rowsum)      (ACT, scale AP)
"""

import numpy as np

import concourse.bass as bass
import concourse.tile as tile
from concourse import bacc, mybir
from concourse.bass_utils import run_bass_kernel_spmd

C = 256
HALF = 128          # output channels per core
N_CORES = 8
ICHUNK = 8          # i values per phase-B chunk
OGRP = 4            # i values per batched-sigmoid/output group
A_CHUNK = 1024      # flattened spatial elems per phase-A chunk

_CACHE = {}


def build_nc(repeat=1):
    key = ("nc", repeat)
    if key in _CACHE:
        return _CACHE[key]
    f32 = mybir.dt.float32
    f32r = mybir.dt.float32r
    f16 = mybir.dt.float16
    bf16 = mybir.dt.bfloat16
    Ident = mybir.ActivationFunctionType.Identity

    nc = bacc.Bacc("TRN2", target_bir_lowering=False, debug=False,
                   num_devices=N_CORES)

    query_b = nc.dram_tensor("query_b", [C, C, C], f32, kind="ExternalInput").ap()
    key_h = nc.dram_tensor("key_h", [C, HALF, C], f32, kind="ExternalInput").ap()
    value_b = nc.dram_tensor("value_b", [C, C, C], f32, kind="ExternalInput").ap()
    wqT = nc.dram_tensor("wqT", [C, HALF], f32, kind="ExternalInput").ap()
    wkT = nc.dram_tensor("wkT", [C, C], f32, kind="ExternalInput").ap()
    wvT = nc.dram_tensor("wvT", [C, HALF], f32, kind="ExternalInput").ap()
    bq_h = nc.dram_tensor("bq_h", [HALF, 1], f32, kind="ExternalInput").ap()
    bk_f = nc.dram_tensor("bk_f", [C, 1], f32, kind="ExternalInput").ap()
    bv_h = nc.dram_tensor("bv_h", [HALF, 1], f32, kind="ExternalInput").ap()
    out_b = nc.dram_tensor("out_b", [HALF, C, C], f32, kind="ExternalOutput").ap()

    q_scr = nc.dram_tensor("q_scr", [HALF, C, C], f16).ap()
    v_scr = nc.dram_tensor("v_scr", [HALF, C, C], bf16).ap()

    HJ = C * C  # 65536
    KCH = ICHUNK * C  # flattened (il, w) per phase-B chunk

    qv_in = query_b.rearrange("(cb c) h j -> c cb (h j)", c=128)
    vv_in = value_b.rearrange("(cb c) j w -> c cb (j w)", c=128)
    kv_in = key_h.rearrange("(cb c) il w -> c cb (il w)", c=128)
    q_flat = q_scr.rearrange("i h j -> i (h j)")
    v_flat = v_scr.rearrange("i j w -> i (j w)")
    out_v = out_b.rearrange("il (hb h) w -> il h hb w", h=128)

    with tile.TileContext(nc) as tc:
        with (
            tc.tile_pool(name="weights", bufs=1) as wpool,
            tc.tile_pool(name="a_in", bufs=3) as a_in,
            tc.tile_pool(name="a_st", bufs=3) as a_st,
            tc.tile_pool(name="ps512", bufs=2, space="PSUM") as ps512,
            tc.tile_pool(name="b_kin", bufs=2) as b_kin,
            tc.tile_pool(name="b_ksb", bufs=2) as b_ksb,
            tc.tile_pool(name="b_qt", bufs=2) as b_qt,
            tc.tile_pool(name="b_vt", bufs=3) as b_vt,
            tc.tile_pool(name="b_et", bufs=3) as b_et,
            tc.tile_pool(name="b_ob", bufs=2) as b_ob,
            tc.tile_pool(name="b_rs", bufs=8) as b_rs,
            tc.tile_pool(name="b_psc", bufs=2, space="PSUM") as b_psc,
            tc.tile_pool(name="b_po", bufs=4, space="PSUM") as b_po,
        ):
            wq_r = wpool.tile([128, 2, HALF], f32r)
            wk_r = wpool.tile([128, 2, C], f32r)
            wv_bf = wpool.tile([128, 2, HALF], bf16)
            nc.gpsimd.dma_start(out=wq_r, in_=wqT.rearrange("(cb c) i -> c cb i", c=128))
            nc.gpsimd.dma_start(out=wk_r, in_=wkT.rearrange("(cb c) j -> c cb j", c=128))
            nc.gpsimd.dma_start(out=wv_bf, in_=wvT.rearrange("(cb c) i -> c cb i", c=128))
            sb_bq = wpool.tile([128, 1], f32)
            sb_bk = wpool.tile([128, 2, 1], f32)
            sb_bv = wpool.tile([128, 1], f32)
            nc.gpsimd.dma_start(out=sb_bq, in_=bq_h)
            nc.gpsimd.dma_start(out=sb_bk, in_=bk_f.rearrange("(jb j) one -> j jb one", j=128))
            nc.gpsimd.dma_start(out=sb_bv, in_=bv_h)

            def body(_it=None):
                # ---------------- Phase A: q and v convs -> DRAM scratch ------
                for t in range(HJ // A_CHUNK):
                    sl = slice(t * A_CHUNK, (t + 1) * A_CHUNK)
                    qc = a_in.tile([128, 2, A_CHUNK], f32r, tag="qc")
                    nc.gpsimd.dma_start(out=qc, in_=qv_in[:, :, sl])
                    qs = a_st.tile([128, A_CHUNK], f16, tag="qs")
                    for n in range(A_CHUNK // 512):
                        ps = ps512.tile([128, 512], f32, tag="aps")
                        for cb in range(2):
                            nc.tensor.matmul(ps, wq_r[:, cb, :],
                                             qc[:, cb, n * 512:(n + 1) * 512],
                                             start=(cb == 0), stop=(cb == 1))
                        nc.vector.tensor_scalar(
                            out=qs[:, n * 512:(n + 1) * 512], in0=ps,
                            scalar1=sb_bq, scalar2=None,
                            op0=mybir.AluOpType.add)
                    nc.sync.dma_start(out=q_flat[:, sl], in_=qs)
                    vc = a_in.tile([128, 2, A_CHUNK], bf16, tag="vc")
                    nc.gpsimd.dma_start(out=vc, in_=vv_in[:, :, sl])
                    vs = a_st.tile([128, A_CHUNK], bf16, tag="vs")
                    for n in range(A_CHUNK // 512):
                        ps = ps512.tile([128, 512], f32, tag="aps")
                        for cb in range(2):
                            nc.tensor.matmul(ps, wv_bf[:, cb, :],
                                             vc[:, cb, n * 512:(n + 1) * 512],
                                             start=(cb == 0), stop=(cb == 1))
                        nc.vector.tensor_scalar(
                            out=vs[:, n * 512:(n + 1) * 512], in0=ps,
                            scalar1=sb_bv, scalar2=None,
                            op0=mybir.AluOpType.add)
                    nc.sync.dma_start(out=v_flat[:, sl], in_=vs)

                # ---------------- Phase B: k conv + attention -----------------
                for ic in range(HALF // ICHUNK):
                    kc = b_kin.tile([128, 2, KCH], f32r, tag="kc")
                    nc.gpsimd.dma_start(
                        out=kc, in_=kv_in[:, :, ic * KCH:(ic + 1) * KCH])
                    ksb = b_ksb.tile([128, 2, ICHUNK, C], f16, tag="ksb")
                    for jb in range(2):
                        for n in range(KCH // 512):
                            ps = ps512.tile([128, 512], f32, tag="aps")
                            for cb in range(2):
                                nc.tensor.matmul(
                                    ps, wk_r[:, cb, jb * 128:(jb + 1) * 128],
                                    kc[:, cb, n * 512:(n + 1) * 512],
                                    start=(cb == 0), stop=(cb == 1))
                            nc.vector.tensor_scalar(
                                out=ksb[:, jb, n * 2:(n + 1) * 2, :], in0=ps,
                                scalar1=sb_bk[:, jb, :], scalar2=None,
                                op0=mybir.AluOpType.add)
                    qt = b_qt.tile([128, ICHUNK, 2, C], f16, tag="qt")
                    for t in range(ICHUNK):
                        i_loc = ic * ICHUNK + t
                        for jb in range(2):
                            nc.sync.dma_start(
                                out=qt[:, t, jb, :],
                                in_=q_scr[i_loc, :, jb * 128:(jb + 1) * 128],
                                transpose=True)
                    for grp in range(ICHUNK // OGRP):
                        # per-group output stage: [h, io, hb, w]
                        ob = b_ob.tile([128, OGRP, 2, C], f32, tag="ob")
                        for io in range(OGRP):
                            t = grp * OGRP + io
                            i_loc = ic * ICHUNK + t
                            vt = b_vt.tile([128, 2, C + 1], bf16, tag="vt")
                            nc.sync.dma_start(
                                out=vt[:, :, 0:C],
                                in_=v_scr[i_loc].rearrange("(jb j) w -> j jb w", j=128))
                            nc.vector.memset(vt[:, :, C:C + 1], 1.0)
                            psc = b_psc.tile([128, 2, C], f32, tag="psc")
                            for wb in range(2):
                                for jb in range(2):
                                    nc.tensor.matmul(
                                        psc[:, wb, :],
                                        ksb[:, jb, t, wb * 128:(wb + 1) * 128],
                                        qt[:, t, jb, :],
                                        start=(jb == 0), stop=(jb == 1))
                            et = b_et.tile([128, 2, C], bf16, tag="et")
                            for wb in range(2):
                                nc.scalar.activation(
                                    out=et[:, wb, :], in_=psc[:, wb, :],
                                    func=mybir.ActivationFunctionType.Exp)
                            for hb in range(2):
                                po = b_po.tile([128, C + 1], f32, tag="po")
                                for wb in range(2):
                                    nc.tensor.matmul(
                                        po, et[:, wb, hb * 128:(hb + 1) * 128],
                                        vt[:, wb, :],
                                        start=(wb == 0), stop=(wb == 1))
                                rs = b_rs.tile([128, 1], f32, tag="rs")
                                nc.vector.reciprocal(out=rs, in_=po[:, C:C + 1])
                                # scale by 1/rowsum on DVE; sigmoid batched below
                                nc.vector.tensor_scalar(
                                    out=ob[:, io, hb, :], in0=po[:, 0:C],
                                    scalar1=rs, scalar2=None,
                                    op0=mybir.AluOpType.mult)
                        # batched sigmoid (single ACT func per group) in-place
                        nc.scalar.activation(
                            out=ob, in_=ob,
                            func=mybir.ActivationFunctionType.Sigmoid)
                        i0 = ic * ICHUNK + grp * OGRP
                        nc.sync.dma_start(
                            out=out_b[i0:i0 + OGRP].rearrange(
                                "io (hb h) w -> h io hb w", h=128),
                            in_=ob)

            if repeat == 1:
                body()
            else:
                with tc.For_i(0, repeat, 1) as it:
                    body(it)

    nc.compile()
    _CACHE[key] = nc
    return nc


def make_in_maps(inputs):
    query = np.ascontiguousarray(inputs["query"], dtype=np.float32)
    key_in = np.ascontiguousarray(inputs["key_in"], dtype=np.float32)
    value = np.ascontiguousarray(inputs["value"], dtype=np.float32)
    Wq = np.asarray(inputs["Wq"], dtype=np.float32)
    Wk = np.asarray(inputs["Wk"], dtype=np.float32)
    Wv = np.asarray(inputs["Wv"], dtype=np.float32)
    bq = np.asarray(inputs["bq"], dtype=np.float32)
    bk = np.asarray(inputs["bk"], dtype=np.float32)
    bv = np.asarray(inputs["bv"], dtype=np.float32)
    in_maps = []
    for core in range(N_CORES):
        b, g = core // 2, core % 2
        sl = slice(g * HALF, (g + 1) * HALF)
        in_maps.append({
            "query_b": query[b],
            "key_h": np.ascontiguousarray(key_in[b][:, sl, :]),
            "value_b": value[b],
            "wqT": np.ascontiguousarray(Wq[sl, :].T),
            "wkT": np.ascontiguousarray(Wk.T),
            "wvT": np.ascontiguousarray(Wv[sl, :].T),
            "bq_h": np.ascontiguousarray(bq[sl].reshape(HALF, 1)),
            "bk_f": np.ascontiguousarray(bk.reshape(C, 1)),
            "bv_h": np.ascontiguousarray(bv[sl].reshape(HALF, 1)),
        })
    return in_maps


def kernel(query, key_in, value, Wq, bq, Wk, bk, Wv, bv):
    nc = build_nc()
    in_maps = make_in_maps(dict(query=query, key_in=key_in, value=value,
                                Wq=Wq, bq=bq, Wk=Wk, bk=bk, Wv=Wv, bv=bv))
    res = run_bass_kernel_spmd(nc, in_maps, core_ids=list(range(N_CORES)))
    out = np.empty((4, C, C, C), dtype=np.float32)
    for core in range(N_CORES):
        b, g = core // 2, core % 2
        out[b, g * HALF:(g + 1) * HALF] = res.results[core]["out_b"]
    return out
